# revision 1
# baseline (speedup 1.0000x reference)
"""Trainium2 Bass kernel for nn_CliffordSirenLayer.

Computes, for full inputs (B=4, N=8192, M=512, IN=OUT=32):
    wT  = einsum('oid,cdk->oick', nan_to_num(weight), CLIFFORD_T)
    pre = einsum('bnic,oick->bnok', x, wT) + bias
    h   = softplus(q @ fw1.T + fb1); ls = clip(h @ fw2.T + fb2, 0, 5)
    dmin = min_m |q - atoms_m| (clamped); omega = 30*(1 + ls*exp(-dmin))
    out = sin(omega * pre)

Sharding: 8 cores; core c handles batch b=c//2, point half c%2 (4096 points).
All parameters are tiny and replicated; everything is embarrassingly parallel.

Device strategy per core (4096 pts = 32 chunks of 128 partitions):
  - Clifford linear as a dense [256,256] matmul, folded on host, run as two
    f32r (fast fp32, 1 cyc/row) K-tile matmuls per chunk into paired PSUM
    tiles (HW-validated: rel err ~1.2e-2 < 2e-2 gate).
  - Distances via |a|^2 - 2q.a + |q|^2: one K=4 f32r matmul per chunk
    against atom features [-2a; |a|^2] (4-way row-group packed), then a
    direct DVE tensor_reduce min over the [128, 512] PSUM tile per chunk.
  - freq-net h via K=4 matmuls sharing the distance stationary operand into
    4 per-row-group PSUM banks; softplus = Ln(Exp(h)+1) on ACT; the fw2
    contraction and clips run on the otherwise-idle Pool (GPSIMD) engine.
  - omega per 8-chunk quarter so phase B pipelines behind the min-reduce
    chain: dist = ACT Sqrt, e = ACT Exp(-dist), the rest on Pool.
  - sin range reduction (ACT Sin diverges past ~pi; no mod/pow ALU on HW):
    t = om2p*pre via ACT Copy(scale) out of PSUM, k = int32(t)
    round-to-nearest on DVE, r = t - k in [-0.5, 0.5], osb = Sin(2*pi*r)
    batched over full 8-chunk groups, written in bf16 (f32 on host).
  - ACT table sets load exactly 3x per rep ([sqrt] -> [exp/ln] -> [sin]),
    enforced with nosync queue-ordering deps, incl. across reps.
  - input x f32 + output y bf16 DMAs split across the SP and Pool queues;
    next-rep inputs prefetch ahead of this rep's output DMAs (the in-order
    queues would otherwise head-of-line block them).
"""

import sys

for _p in ("/opt/trn_rl_repo", "/root/.axon_site/_ro/trn_rl_repo"):
    if _p not in sys.path:
        sys.path.append(_p)

import numpy as np
import ml_dtypes

import concourse.bass as bass
from concourse.instruction_name_ordered_set import InstructionNameOrderedSet
import concourse.bass_isa as bass_isa
import concourse.tile as tile
from concourse import bacc, mybir
from concourse.bass_utils import run_bass_kernel_spmd

F32 = mybir.dt.float32
F32R = mybir.dt.float32r
I32 = mybir.dt.int32
BF16 = mybir.dt.bfloat16
AF = mybir.ActivationFunctionType
ALU = mybir.AluOpType
AX = mybir.AxisListType

B, N, M, IN, OUT = 4, 8192, 512, 32, 32
NCORES = 8
NLOC = (B * N) // NCORES          # 4096 points per core
CH = 128                          # points per chunk (partition dim)
NCH = NLOC // CH                  # 32 chunks
D = IN * 8                        # 256 contraction dim
DO = OUT * 8                      # 256 output dim

TWO_PI = 6.283185307179586
PI = 3.141592653589793
INV_2PI = 0.15915494309189535
SIREN_OMEGA_0 = 30.0
OM_SCALE = SIREN_OMEGA_0 * INV_2PI

# exp(-dist) on Pool: e = (2^(-t/8))^8 with t/8 = sqrt(max(d2,1e-4)*C8E),
# 2^(-v) as a degree-6 polynomial on v in [0, 1.6] (abs err < 3e-8).
C8E = 0.03252196783261596            # (log2(e)/8)^2
E2C = [8.944635143317914e-05, -0.001205020699733234, 0.00948517119716185,
       -0.055431921902920296, 0.24020764956757276, -0.6931453225050171,
       0.9999999700269586]

def _perm(t):
    """Column index for chunk t in the omega tiles: 16*half + 4*g + (tg%4),
    so half-h slices are contiguous 16-col blocks and quarter-q slices are
    [4,2]-strided blocks."""
    g, tg = t % 4, t // 4
    return 16 * (tg // 4) + 4 * g + (tg % 4)


def _clifford_table():
    masks = [0, 1, 2, 4, 3, 5, 6, 7]
    idx = {m: i for i, m in enumerate(masks)}
    T = np.zeros((8, 8, 8), np.float64)
    for i, a in enumerate(masks):
        for j, b in enumerate(masks):
            s, aa = 1, a >> 1
            while aa:
                if bin(aa & b).count("1") & 1:
                    s = -s
                aa >>= 1
            T[i, j, idx[a ^ b]] = s
    return T


def build_program(with_bias: bool, reps: int = 1):
    """Build + compile the per-core SPMD bass program."""
    nc = bacc.Bacc("TRN2", target_bir_lowering=False, debug=False, num_devices=1)

    dram = {
        "x0": nc.dram_tensor("x0", [CH, NLOC], F32R, kind="ExternalInput").ap(),
        "x1": nc.dram_tensor("x1", [CH, NLOC], F32R, kind="ExternalInput").ap(),
        "wm": nc.dram_tensor("wm", [CH, 2 * DO], F32R, kind="ExternalInput").ap(),
        "qt4": nc.dram_tensor("qt4", [128, (NCH // 4) * CH], F32R,
                              kind="ExternalInput").ap(),
        # f32r pack: [af4 (512) | fw1f4 (16)]
        "pkr": nc.dram_tensor("pkr", [128, M + 16], F32R,
                              kind="ExternalInput").ap(),
        # f32 pack: [fw2rp (512) | q2tp (32) | fb2s (1)]
        "pk": nc.dram_tensor("pk", [128, NCH * 16 + NCH + 1], F32,
                             kind="ExternalInput").ap(),
    }
    if with_bias:
        dram["brow"] = nc.dram_tensor("brow", [1, DO], F32R,
                                      kind="ExternalInput").ap()
    Y = nc.dram_tensor("y", [NLOC, DO], BF16, kind="ExternalOutput").ap()

    with tile.TileContext(nc) as tc:
        with (
            tc.tile_pool(name="const", bufs=2) as cp,
            tc.tile_pool(name="xin", bufs=2) as xp,
            tc.tile_pool(name="work", bufs=2) as wp,
            tc.tile_pool(name="scr", bufs=3) as scp,
            tc.tile_pool(name="rrp", bufs=3) as rp,
            tc.tile_pool(name="tmp", bufs=3) as tp,
            tc.tile_pool(name="kip", bufs=2) as kp,
            tc.tile_pool(name="outp", bufs=4) as op,
            tc.tile_pool(name="psA", bufs=2, space="PSUM") as psA,
            tc.tile_pool(name="psH", bufs=1, space="PSUM") as psH,
            tc.tile_pool(name="psB", bufs=2, space="PSUM") as psB,
        ):
            P = dict(cp=cp, xp=xp, wp=wp, scp=scp, rp=rp, tp=tp, kp=kp,
                     op=op, psA=psA, psH=psH, psB=psB)
            state = None
            for i in range(reps):
                state = _emit_body(nc, P, dram, Y, with_bias, state,
                                   prefetch=(i + 1 < reps))

    nc.compile()
    return nc


def _emit_input_dmas(nc, P, dram, with_bias):
    """Allocate input tiles and issue their DMAs (SP + Pool queues).
    Called inline for the first rep and as a prefetch for rep n+1 from
    inside rep n (before rep n's y DMAs, so the next rep's inputs are
    never head-of-line blocked behind them)."""
    cp, xp = P["cp"], P["xp"]
    inp = {}
    inp["pkr"] = cp.tile([128, M + 16], F32R, tag="pkr", name="pkr")
    nc.sync.dma_start(inp["pkr"][:], dram["pkr"][:])
    qt4 = cp.tile([128, (NCH // 4) * CH], F32R, tag="qt4")
    hc = (NCH // 8) * CH
    nc.sync.dma_start(qt4[:, 0:hc], dram["qt4"][:, 0:hc])
    nc.sync.dma_start(qt4[:, hc:], dram["qt4"][:, hc:])
    inp["qt4"] = qt4
    inp["pk"] = cp.tile([128, NCH * 16 + NCH + 1], F32, tag="pk", name="pk")
    nc.sync.dma_start(inp["pk"][:], dram["pk"][:])
    inp["wm"] = cp.tile([128, 2 * DO], F32R, tag="wm", name="wm")
    nc.gpsimd.dma_start(inp["wm"][:], dram["wm"][:])
    x0 = xp.tile([CH, NLOC], F32R, tag="x0", name="x0")
    x1 = xp.tile([CH, NLOC], F32R, tag="x1", name="x1")
    QR = NLOC // 4
    for r in range(4):
        sl = bass.ts(r, QR)
        e0 = nc.gpsimd if r < 2 else nc.sync
        e0.dma_start(x0[:, sl], dram["x0"][:, sl])
        e1 = nc.gpsimd if r == 2 else nc.sync
        e1.dma_start(x1[:, sl], dram["x1"][:, sl])
    inp["x0"], inp["x1"] = x0, x1
    if with_bias:
        inp["ones1"] = cp.tile([1, CH], F32R, tag="ones", name="ones1")
        nc.vector.memset(inp["ones1"][:], 1.0)
        inp["brow"] = cp.tile([1, DO], F32R, tag="brow", name="brow")
        nc.sync.dma_start(inp["brow"][:], dram["brow"][:])
    return inp


def _emit_body(nc, P, dram, Y, with_bias, state, prefetch):
    """Emit one rep. state = (act_gate, inp) from the previous rep: act_gate
    is the previous rep's last Sin instruction name (this rep's Exp/Ln ACT
    ops are queue-ordered after it so each activation table set loads
    exactly once per rep); inp holds this rep's prefetched input tiles.
    Emission interleaves phase-A chunk blocks with phase-B groups so the PE
    queue never parks phase-B matmuls behind the whole distance sweep.
    Returns (last_sin_name, next_inp)."""
    cp, wp, scp, rp, tp, op = (P["cp"], P["wp"], P["scp"], P["rp"], P["tp"],
                               P["op"])
    kp = P["kp"]
    psA, psH, psB = P["psA"], P["psH"], P["psB"]
    NG = NCH // 4                 # chunks per column block of qt4 (8)

    if state is None:
        act_gate, inp = None, _emit_input_dmas(nc, P, dram, with_bias)
    else:
        act_gate, inp = state
    qt4, pkr, pk, wm = inp["qt4"], inp["pkr"], inp["pk"], inp["wm"]
    x0, x1 = inp["x0"], inp["x1"]
    af4 = pkr[:, 0:M]
    fw1f4 = pkr[:, M:M + 16]
    fw2rp = pk[:, 0:NCH * 16]
    q2tp = pk[:, NCH * 16:NCH * 16 + NCH]
    fb2s = pk[:, NCH * 16 + NCH:NCH * 16 + NCH + 1]

    exp_block = []                # Exp/Ln instructions (table set 6)
    sqrt_block = []               # Sqrt instructions (table set 3)

    def gated(binst):
        exp_block.append(binst)
        return binst

    def add_dep(binst, name):
        s = InstructionNameOrderedSet()
        s.add(name)
        tgt = binst if hasattr(binst, "add_nosync_dependencies_from") \
            else binst.ins
        tgt.add_nosync_dependencies_from(s)

    pib = cp.tile([128, 1], F32, tag="pib")
    nc.vector.memset(pib[:], PI)

    # Preload the exp+ln table set so the auto-inserter doesn't pick
    # exp_and_others (no ln) and ping-pong sets mid-kernel. Gated after
    # the previous rep's sins like the rest of the exp block.
    atl = mybir.InstLoadActFuncSet(
        name=nc.get_next_instruction_name(),
        ins=[], outs=[],
        act_func_set_id=6,  # natural_log_exp_and_others
    )
    nc.scalar.add_instruction(atl)

    # ---- persistent tiles for this rep ----
    dminP = cp.tile([128, NCH], F32, tag="dminP")
    h_ps = [psH.tile([128, NG * 16], F32, tag=f"hps{g}", name=f"hps{g}")
            for g in range(4)]
    he = wp.tile([128, NCH * 16], F32, tag="he")
    hsp = wp.tile([128, NCH * 16], F32, tag="hsp")
    lsr = cp.tile([128, NCH], F32, tag="lsr")
    ls = cp.tile([128, NCH], F32, tag="ls")
    d2c = cp.tile([128, NCH], F32, tag="d2c")
    ve = cp.tile([128, NCH], F32, tag="ve")
    e = cp.tile([128, NCH], F32, tag="e")
    om2p = cp.tile([128, NCH], F32, tag="om2p")    # omega / (2*pi)
    HB = (NCH // 2) * 16          # 256 he/hsp cols per half

    def phase_a_block(lo, hi):
        """d2 + h matmuls and the fused min for chunks [lo, hi)."""
        for t in range(lo, hi):
            g, tg = t % 4, t // 4
            lhs = qt4[32 * g:32 * g + 4, bass.ts(tg, CH)]
            d2 = psA.tile([128, M], F32, tag="d2")
            nc.tensor.matmul(d2[:], lhs, af4[32 * g:32 * g + 4, :],
                             start=True, stop=True,
                             tile_position=(32 * g, 0))
            nc.tensor.matmul(h_ps[g][:, 16 * tg:16 * (tg + 1)], lhs,
                             fw1f4[32 * g:32 * g + 4, :],
                             start=True, stop=True,
                             tile_position=(32 * g, 0))
            pcol = _perm(t)
            if True:
                # direct single-input min-reduce (DVE) for 4 of 32 chunks
                nc.vector.tensor_reduce(
                    dminP[:, pcol:pcol + 1],
                    d2[:].rearrange("p (u a) -> p u a", u=1),
                    axis=AX.X, op=ALU.min)
            else:
                # ACT copies the upper atom half to SBUF so the fused
                # min+reduce reads one PSUM and one SBUF operand (HW rule)
                sc2 = scp.tile([128, M // 2], F32, tag="scr")
                nc.scalar.activation(sc2[:], d2[:, M // 2:M], AF.Copy)
                scr = scp.tile([128, M // 2], F32, tag="scro")
                nc.vector.tensor_tensor_reduce(
                    scr[:], d2[:, 0:M // 2], sc2[:], 1.0, 1e30,
                    ALU.min, ALU.min, dminP[:, pcol:pcol + 1])

    def local_scale_half(h):
        """ls for half h (softplus on ACT + fw2 contraction on Pool)."""
        for g in range(4):
            gated(nc.scalar.activation(
                he[:, HB * h + 64 * g:HB * h + 64 * (g + 1)],
                h_ps[g][:, 64 * h:64 * (h + 1)], AF.Exp))
        hs = slice(HB * h, HB * (h + 1))
        gated(nc.scalar.activation(hsp[:, hs], he[:, hs], AF.Ln, bias=1.0))
        prod = wp.tile([128, HB], F32, tag=f"prod{h}")
        nc.gpsimd.tensor_mul(prod[:], hsp[:, hs], fw2rp[:, 0:HB])
        # segmented sum over j=16 as a Pool add-tree (keeps DVE free)
        p3 = prod[:].rearrange("p (t j) -> p t j", j=16)
        nc.gpsimd.tensor_add(p3[:, :, 0:8], p3[:, :, 0:8], p3[:, :, 8:16])
        nc.gpsimd.tensor_add(p3[:, :, 0:4], p3[:, :, 0:4], p3[:, :, 4:8])
        nc.gpsimd.tensor_add(p3[:, :, 0:2], p3[:, :, 0:2], p3[:, :, 2:4])
        cs = slice(16 * h, 16 * (h + 1))
        nc.gpsimd.tensor_add(lsr[:, cs].rearrange("p (t j) -> p t j", j=1),
                             p3[:, :, 0:1], p3[:, :, 1:2])
        nc.gpsimd.tensor_scalar(ls[:, cs], lsr[:, cs], fb2s, 0.0,
                                ALU.add, ALU.max)
        nc.gpsimd.tensor_scalar_min(ls[:, cs], ls[:, cs], 5.0)

    def omega_quarter(q):
        """omega for chunks 8q..8q+7: dist = sqrt(clamp(d2c)), e = exp(-dist),
        om2p = OM_SCALE*(1 + ls*e). Sqrt lives in table set 3; Exp in set 6;
        the *_block lists get queue-ordered into [sqrt][exp/ln][sin] stretches
        after emission."""
        def qs(ap):
            # [128, 4, 2] strided view of quarter q's 8 columns
            return ap[:, 16 * (q // 2):16 * (q // 2) + 16].rearrange(
                "p (g r) -> p g r", r=4)[:, :, 2 * (q % 2):2 * (q % 2) + 2]

        nc.gpsimd.tensor_add(qs(d2c), qs(dminP), qs(q2tp))
        nc.gpsimd.tensor_scalar_max(qs(d2c), qs(d2c), 1e-4)
        sqrt_block.append(nc.scalar.activation(qs(ve), qs(d2c), AF.Sqrt))
        exp_block.append(nc.scalar.activation(qs(e), qs(ve), AF.Exp,
                                              scale=-1.0))
        nc.gpsimd.tensor_mul(qs(om2p), qs(ls), qs(e))
        nc.gpsimd.tensor_scalar(qs(om2p), qs(om2p), OM_SCALE, OM_SCALE,
                                ALU.mult, ALU.add)

    last_sin = [None]
    sin_list = []

    def phase_b_group(j):
        """Clifford matmuls + modulated sin for chunks 8j..8j+7.
        t = om2p*pre; k = round-to-nearest-int(t); r = t - k in [-0.5, 0.5];
        osb = Sin(2pi*r)."""
        rr = rp.tile([128, 8, DO], F32, tag="rr")
        tm = tp.tile([128, 8, DO], F32, tag="tm")
        ki = kp.tile([128, 8, DO], I32, tag="ki")
        osb = op.tile([128, 8, DO], BF16, tag="osb")
        for tp2 in range(4):
            pre2 = psB.tile([128, 2, DO], F32, tag="pre")
            for i in range(2):
                tt = 2 * tp2 + i
                t = 8 * j + tt
                pcol = _perm(t)
                om_v = om2p[:, pcol:pcol + 1]
                pre = pre2[:, i, :]
                nc.tensor.matmul(pre, x0[:, bass.ts(t, CH)], wm[:, 0:DO],
                                 start=True, stop=False)
                nc.tensor.matmul(pre, x1[:, bass.ts(t, CH)],
                                 wm[:, DO:2 * DO], start=False,
                                 stop=not with_bias)
                if with_bias:
                    nc.tensor.matmul(pre, inp["ones1"][:], inp["brow"][:],
                                     start=False, stop=True)
                # ACT applies the omega scale on the way out of PSUM
                # (keeps the DVE free for the min-reduce chain)
                nc.scalar.activation(tm[:, tt, :], pre, AF.Copy,
                                     scale=om_v)
        # round-to-nearest int on Pool (frees the DVE for the min-reduce
        # chain); subtract stays on DVE, batched over the whole group
        nc.gpsimd.tensor_copy(ki[:], tm[:])
        nc.vector.tensor_sub(rr[:], tm[:], ki[:])
        sin_inst = nc.scalar.activation(osb[:], rr[:], AF.Sin, scale=TWO_PI)
        sin_list.append(sin_inst)
        last_sin[0] = sin_inst

        dst = Y[1024 * j:1024 * (j + 1), :].rearrange("(c p) o -> p c o",
                                                      p=128)
        nc.sync.dma_start(dst, osb[:])

    # ---- interleaved emission: all phase-A ACT copies and the full
    # exp/ln block precede every sin in the ACT queue ----
    phase_a_block(0, 8)
    phase_a_block(8, 16)
    phase_a_block(16, 24)
    local_scale_half(0)
    omega_quarter(0)
    # prefetch next rep's inputs here: ahead of this rep's y DMAs in the
    # SP/Pool queue order, so they are never head-of-line blocked
    next_inp = _emit_input_dmas(nc, P, dram, with_bias) if prefetch else None
    phase_a_block(24, 32)
    local_scale_half(1)
    phase_b_group(0)
    omega_quarter(1)
    phase_b_group(1)
    omega_quarter(2)
    phase_b_group(2)
    omega_quarter(3)
    phase_b_group(3)

    # ---- ACT-queue set ordering: [sqrt x4][set6: he/hsp/e][sins] ----
    if act_gate is not None:
        for b in sqrt_block:
            add_dep(b, act_gate)
    add_dep(atl, sqrt_block[-1].ins.name)
    for b in exp_block:
        add_dep(b, sqrt_block[-1].ins.name)
    for b in sin_list:
        add_dep(b, exp_block[-1].ins.name)

    return (last_sin[0].ins.name, next_inp)


def prepare_inputs(x, query_coords, atomic_coords, weight, bias, fw1, fb1, fw2, fb2):
    """Host-side prep: fold the Clifford table into W, pack per-core layouts."""
    T = _clifford_table()
    w64 = np.nan_to_num(np.asarray(weight)).astype(np.float64)
    Wm = np.einsum("oid,cdk->icok", w64, T).reshape(D, DO).astype(np.float32)
    wm = np.ascontiguousarray(np.concatenate([Wm[0:CH, :], Wm[CH:D, :]], axis=1))

    bias_flat = np.asarray(bias).astype(np.float32).reshape(DO)
    with_bias = bool(np.any(bias_flat))
    brow = bias_flat.reshape(1, DO).copy()

    fw1 = np.asarray(fw1).astype(np.float64)
    fb1 = np.asarray(fb1).astype(np.float64)
    fw2 = np.asarray(fw2).astype(np.float64).reshape(16)
    fb2 = float(np.asarray(fb2).reshape(()))

    fw1_feat = np.concatenate([fw1.T, fb1.reshape(1, 16)], axis=0)  # [4,16]
    fw1f4 = np.zeros((128, 16), np.float32)
    for g in range(4):
        fw1f4[32 * g:32 * g + 4, :] = fw1_feat
    fw2rp = np.tile(fw2.astype(np.float32), (128, NCH))             # [128, 512]
    fb2s = np.full((128, 1), fb2, np.float32)

    x = np.asarray(x)
    q_all = np.asarray(query_coords).astype(np.float64)
    a_all = np.asarray(atomic_coords).astype(np.float64)
    NG = NCH // 4

    in_maps = []
    for c in range(NCORES):
        b, half = c // 2, c % 2
        sl = slice(half * NLOC, (half + 1) * NLOC)
        xT = np.ascontiguousarray(x[b, sl].reshape(NLOC, D).T.astype(np.float32))

        q = q_all[b, sl]                                            # [4096, 3]
        paug = np.concatenate([q.T, np.ones((1, NLOC))], axis=0)    # [4, 4096]
        pa = paug.reshape(4, NG, 4, CH)                             # [k, tg, g, j]
        qt4 = np.zeros((128, NG * CH), np.float32)
        for g in range(4):
            for k in range(4):
                qt4[32 * g + k, :] = pa[k, :, g, :].reshape(-1)

        a = a_all[b]                                                # [512, 3]
        feat = np.concatenate([-2.0 * a.T, (a * a).sum(1).reshape(1, M)], axis=0)
        af4 = np.zeros((128, M), np.float32)
        for g in range(4):
            af4[32 * g:32 * g + 4, :] = feat
        pkr = np.concatenate([af4, fw1f4], axis=1)                  # [128, 528]

        q2 = (q * q).sum(1).astype(np.float32)                      # [4096]
        q2t = q2.reshape(NCH, CH).T                                 # [128, 32]
        q2tp = np.zeros((128, NCH), np.float32)
        for t in range(NCH):
            q2tp[:, _perm(t)] = q2t[:, t]
        pk = np.concatenate([fw2rp, q2tp, fb2s], axis=1)            # [128, 545]

        m = {
            "x0": xT[0:CH], "x1": xT[CH:D], "wm": wm,
            "qt4": qt4, "pkr": pkr, "pk": pk,
        }
        if with_bias:
            m["brow"] = brow
        in_maps.append(m)
    return in_maps, with_bias


_PROGRAM_CACHE = {}


def get_program(with_bias: bool, reps: int = 1):
    key = (with_bias, reps)
    if key not in _PROGRAM_CACHE:
        _PROGRAM_CACHE[key] = build_program(with_bias, reps)
    return _PROGRAM_CACHE[key]


def assemble_output(results):
    out = np.empty((B, N, OUT, 8), np.float32)
    for c in range(NCORES):
        b, half = c // 2, c % 2
        y = np.asarray(results[c]["y"]).astype(np.float32)
        out[b, half * NLOC:(half + 1) * NLOC] = y.reshape(NLOC, OUT, 8)
    return out


def kernel(x, query_coords, atomic_coords, weight, bias, fw1, fb1, fw2, fb2):
    in_maps, with_bias = prepare_inputs(
        x, query_coords, atomic_coords, weight, bias, fw1, fb1, fw2, fb2)
    nc = get_program(with_bias)
    res = run_bass_kernel_spmd(nc, in_maps, core_ids=list(range(NCORES)))
    return assemble_output(res.results)


if __name__ == "__main__":
    print("kernel module loaded; run test.py for the full check")

